# revision 42
# baseline (speedup 1.0000x reference)
"""Self-contained Trainium2 Bass kernel for nn_Decoder_79809082294812.

kernel(**inputs) takes the FULL unsharded inputs (embeddings [1024,1000,128],
remaining_capacity [1024], Wqg [257,128], Wkg/Wvg/Wog/Wqo/Wko [128,128],
current_node [1024], mask [1024,1000]) and returns (probs, logits), each
[1024, 1000] float32 — matching the reference decoder.

Sharding: pure data-parallel over the batch dim across 8 NeuronCores
(128 batch elements per core); weights replicated.

Device pipeline (per core, 8 tiles of 16 batch elements):
  - host precomputes q = context@Wqg and the per-element U matrices
    (U_b = (Wkg/sqrt(D)) @ q_b per head), packed as U32 [E, b, 32] with
    U at column offset 8*(b%4), so 16 elements' compat rows pack densely
    into one [128, 1024] PSUM tile (rows 32*(j//4) + 8*(j%4) + h) via
    accumulating matmuls at 4 tile_positions.
  - softmax without max-subtraction (|compat| < ~8), normalization folded
    into the attention transpose: attnT = exN^T @ diag(recip) as a regular
    matmul with a runtime diagonal moving operand.
  - glimpse accumulation A streams the natural-layout chunks as matmul
    stationaries, interleaved with the next tile's compat matmuls so the
    128-column LDWEIGHTS hide behind 512-column moving matmuls.
  - comp rows for all 128 batch elements accumulate into a single dense
    [128, 1024] PSUM tile (stationary w at column b%32, tile_position
    32*(b//32)), so the tanh/softmax epilogue is 3 dense [128,1000] passes.
  - logits are output as tanh(comp); the *10 scale is applied on host.

DMA strategy (v2): both on-chip layouts are host-pretransposed into
DRAM layouts whose per-partition lines are large and contiguous
(32KB for embT, 16KB for nat), so plain HWDGE dma_start on the sync
ring hits HBM line rate.  The old xbar dma_start_transpose path capped
at ~260GB/s and serialized all 66MB on one ring (~255us).  The nat
stream is additionally fp8 (stationary operand of the A-pass matmuls;
attnT moving stays bf16) halving it to 16.4MB: validated 1.52e-2
worst relerr vs the 2e-2 gate.
"""
import contextlib
import ctypes
import math
import os
import sys
import types

sys.path.insert(0, '/opt/trn_rl_repo')

from contextlib import ExitStack
import numpy as np
import ml_dtypes

import concourse.bass as bass
import concourse.tile as tile
from concourse import bacc, mybir
from concourse.bass_utils import run_bass_kernel_spmd

F32 = mybir.dt.float32
BF16 = mybir.dt.bfloat16
FP8 = mybir.dt.float8e4
AF = mybir.ActivationFunctionType
AX = mybir.AxisListType
ALU = mybir.AluOpType
BF16_NP = ml_dtypes.bfloat16
FP8_NP = ml_dtypes.float8_e4m3fn

B = 1024
N = 1000
E = 128
H = 8
D = 16
N_CORES = 8
BC = B // N_CORES   # batch elements per core
TB = 16             # batch elements per tile
NT = BC // TB       # tiles per core
NCH = 8             # n-chunks (node n lives at chunk n%8, row n//8)
CH = 125            # rows per chunk
SPLIT = 512         # psum-bank-aligned split of the n axis

NAT_FP8 = True      # natural-layout embedding stream dtype (fp8 halves DMA)
EMBT_BUFS = 4       # embT lives 3 iterations (compat@i, comp@i+2) + prefetch
NAT_BUFS = 3

WNAME_SHAPES = {
    "mhcat": ([E, H, E], BF16),
    "identf": ([128, 128], F32),
}
assert B % (N_CORES * TB) == 0 and CH * NCH == N

_NC_CACHE = {}
LAST_RESULT = None   # BassKernelResults of the most recent run (for profiling)


# --------------------------------------------------------------------------
# Optional NTFF profiling hook (enabled only when BASS_TRACE is set).
# --------------------------------------------------------------------------
def _install_profile_shim():
    so_path = '/opt/axon/libaxon_pjrt.so'
    try:
        import antenv
    except ImportError:
        return
    if 'antenv.axon_hooks' not in sys.modules:
        mod = types.ModuleType('antenv.axon_hooks')
        mod._hook = None

        def set_axon_ntff_profile_hook(h):
            mod._hook = h

        def get_axon_ntff_profile_hook():
            return mod._hook

        mod.set_axon_ntff_profile_hook = set_axon_ntff_profile_hook
        mod.get_axon_ntff_profile_hook = get_axon_ntff_profile_hook
        sys.modules['antenv.axon_hooks'] = mod
        antenv.axon_hooks = mod
    mod = sys.modules['antenv.axon_hooks']
    if mod.get_axon_ntff_profile_hook() is not None:
        return
    try:
        lib = ctypes.CDLL(so_path)
    except OSError:
        return
    if not hasattr(lib, "axon_start_nrt_profile"):
        return
    lib.axon_start_nrt_profile.argtypes = [ctypes.POINTER(ctypes.c_int64),
                                           ctypes.c_size_t]
    lib.axon_start_nrt_profile.restype = ctypes.c_int64
    lib.axon_stop_nrt_profile.argtypes = [ctypes.c_char_p]
    lib.axon_stop_nrt_profile.restype = ctypes.c_int64

    @contextlib.contextmanager
    def _hook(output_dir, device_ids):
        import jax
        jax.devices()
        if device_ids:
            ids = (ctypes.c_int64 * len(device_ids))(*device_ids)
            rc = lib.axon_start_nrt_profile(ids, len(device_ids))
        else:
            rc = lib.axon_start_nrt_profile(None, 0)
        if rc != 0:
            raise RuntimeError(f"axon_start_nrt_profile rc={rc}")
        try:
            yield
        finally:
            n = lib.axon_stop_nrt_profile(str(output_dir).encode())
            if n < 0:
                raise RuntimeError(f"axon_stop_nrt_profile rc={n}")

    mod.set_axon_ntff_profile_hook(_hook)
    import concourse.bass_utils as bu
    bu.upload_artifacts = lambda tmpdir: f"local:{tmpdir}"


def _host_prep_weights(Wvg, Wog, Wqo, Wko):
    w = {}
    # fused heads-extraction + output projection: w_b = sum_h M_h @ A_b[:,h]
    # with M_h = wbig^T E_h Wvg^T; the stationary is M_h^T = Wvg E_h wbig
    # = Wvg[:, 16h:16h+16] @ wbig[16h:16h+16, :].
    wbig = (Wog @ Wqo @ Wko.T) / math.sqrt(E)
    mh = np.stack([Wvg[:, D * h:D * h + D] @ wbig[D * h:D * h + D, :]
                   for h in range(H)], axis=1)          # [E, H, E]
    w["mhcat"] = np.ascontiguousarray(mh.astype(BF16_NP))
    w["identf"] = np.eye(128, dtype=np.float32)
    return w


def _host_prep_u32(embeddings, remaining_capacity, Wqg, Wkg, current_node):
    """U32 [E, B, 32] bf16: U for element b at columns 8*(b%4)..+8."""
    graph = embeddings.mean(axis=1)                       # [B, E]
    cur = embeddings[np.arange(B), current_node]          # [B, E]
    context = np.concatenate(
        [graph, cur, remaining_capacity[:, None]], axis=-1)
    q = (context @ Wqg).reshape(B, H, D)
    U = np.einsum('ehd,bhd->ebh',
                  (Wkg / math.sqrt(D)).reshape(E, H, D).astype(np.float32),
                  q.astype(np.float32))                   # [E, B, H]
    # element j = b%16 sits in PE quadrant j%4 at column offset 8*(j//4):
    # the accumulation step q = j//4, so compat steps q0+q1 only need the
    # first half (j<8) of the embT tile DMA.
    U32 = np.zeros((E, B, 32), dtype=BF16_NP)
    off = 8 * ((np.arange(B) // 4) % 4)
    for r in range(4):
        sel = off == 8 * r
        U32[:, sel, 8 * r:8 * r + 8] = U[:, sel, :].astype(BF16_NP)
    return U32, cur


def _build_nc(Bc=BC, n_devices=N_CORES):
    nat_dt, nat_np_elem = (FP8, 1) if NAT_FP8 else (BF16, 2)
    nc = bacc.Bacc("TRN2", target_bir_lowering=False, debug=False,
                   num_devices=n_devices)

    # Both streams are host-pretransposed so a PLAIN dma_start lands the
    # on-chip layout directly: per-partition lines are one contiguous
    # 32KB (embT) / 16KB (nat) DRAM run per tile, so HWDGE emits 128 big
    # descriptors per transfer and runs at HBM line rate (~358GB/s),
    # vs ~260GB/s for the old serialized xbar dma_start_transpose path.
    embtd = nc.dram_tensor("embt", [NT, E, TB, N], BF16,
                           kind="ExternalInput").ap()
    natpd = nc.dram_tensor("natp", [NT, 128, TB, NCH, E], nat_dt,
                           kind="ExternalInput").ap()
    u32d = nc.dram_tensor("u32", [E, Bc * 32], BF16, kind="ExternalInput").ap()
    wap = {k: nc.dram_tensor(k, s, dt, kind="ExternalInput").ap()
           for k, (s, dt) in WNAME_SHAPES.items()}
    probs_out = nc.dram_tensor("probs", [Bc, N], F32, kind="ExternalOutput").ap()
    tanh_out = nc.dram_tensor("tanh", [Bc, N], F32, kind="ExternalOutput").ap()

    with tile.TileContext(nc) as tc, ExitStack() as ctx:
        cpool = ctx.enter_context(tc.tile_pool(name="consts", bufs=1))
        # ---- pools ----
        embT_pool = ctx.enter_context(tc.tile_pool(name="embT", bufs=EMBT_BUFS))
        nat_pool = ctx.enter_context(tc.tile_pool(name="nat", bufs=NAT_BUFS))
        exn_pool = ctx.enter_context(tc.tile_pool(name="exn", bufs=2))
        attnT_pool = ctx.enter_context(tc.tile_pool(name="attnT", bufs=2))
        sm_pool = ctx.enter_context(tc.tile_pool(name="smalls", bufs=2))
        stage_pool = ctx.enter_context(tc.tile_pool(name="stage", bufs=1))

        # PSUM (8 banks): pcm 1x2 + pcomp 1x2 + pat 1x2 + paux 2x1
        pcm_pool = ctx.enter_context(tc.tile_pool(name="pcm", bufs=1, space="PSUM"))
        pcomp_pool = ctx.enter_context(tc.tile_pool(name="pcomp", bufs=1, space="PSUM"))
        pat_pool = ctx.enter_context(tc.tile_pool(name="pat", bufs=1, space="PSUM"))
        paux_pool = ctx.enter_context(tc.tile_pool(name="paux", bufs=2, space="PSUM"))

        pcomp = pcomp_pool.tile([128, 1024], F32, tag="pcomp")
        t_th = stage_pool.tile([128, N], F32, tag="tanh")
        p_stage = stage_pool.tile([128, N], F32, tag="probs")

        # exn is [128, 1024]: cols 1000..1023 are zeroed once per buffer so
        # the pat transposes can use full 128-col stationaries (p=125..127
        # transpose to zero attnT rows) -- 128-col stationaries enable FWL.
        for _ in range(2):
            e = exn_pool.tile([128, NCH * 128], BF16, tag="exn")
            nc.gpsimd.memset(e[:, N:], 0.0)

        def load_tile(t):
            # embT as two SEPARATE half-tiles (Tile tracks DMA deps at
            # tile granularity, so a split DMA into one tile would make
            # readers wait for both halves).  Ring order [a, natp, b]:
            # compat q0/q1 of tile t run on half a one block early, natp
            # lands before the A-pass needs it, and half b's landing is
            # the block clock.  Tile 0 loads [a, b, natp] so its full
            # compat can start ASAP.
            embTa = embT_pool.tile([E, TB // 2, N], BF16, tag="embTa")
            embTb = embT_pool.tile([E, TB // 2, N], BF16, tag="embTb")
            natg = nat_pool.tile([128, TB, NCH, E], nat_dt, tag="nat")
            nc.sync.dma_start(embTa[:], embtd[t, :, :TB // 2])
            if t == 0:
                nc.sync.dma_start(embTb[:], embtd[t, :, TB // 2:])
                nc.sync.dma_start(natg[:], natpd[t])
            else:
                nc.sync.dma_start(natg[:], natpd[t])
                nc.sync.dma_start(embTb[:], embtd[t, :, TB // 2:])
            return embTa, embTb, natg

        def emb_of(s, j):
            return s["embTa"][:, j] if j < TB // 2 else \
                s["embTb"][:, j - TB // 2]

        # ---- constants: u32 + weights go on the gpsimd SWDGE queue so
        # they stream in parallel with embT(0) on the sync ring (the
        # scalar HWDGE ring crawled at ~25GB/s for the 1MB u32; SWDGE
        # measures ~340GB/s at this size).
        preload0 = load_tile(0)
        u32_sb = cpool.tile([E, Bc, 32], BF16, tag="u32")
        nc.gpsimd.dma_start(
            u32_sb[:].rearrange("e b k -> e (b k)"), u32d[:])
        w_sb = {}
        for k, (s, dt) in WNAME_SHAPES.items():
            t = cpool.tile(s, dt, tag=k)
            nc.gpsimd.dma_start(t[:], wap[k][:])
            w_sb[k] = t
        preload1 = load_tile(1)

        def compat_mms(t, s, pcm, qlo, qhi):
            """compat matmuls for accumulation steps qlo..qhi; element
            j = 4q+pp sits in quadrant pp (4-way concurrent).  Steps 0-1
            read only embT half a, steps 2-3 only half b."""
            for q in range(qlo, qhi):
                for pp in range(4):
                    j = 4 * q + pp
                    for s0, s1 in ((0, SPLIT), (SPLIT, N)):
                        nc.tensor.matmul(
                            pcm[32 * pp:32 * pp + 32, s0:s1],
                            u32_sb[:, t * TB + j, :],
                            emb_of(s, j)[:, s0:s1],
                            start=(q == 0), stop=(q == 3),
                            tile_position=(0, 32 * pp))

        def comp_mms(t, s, jlo, jhi):
            """comp matmuls for elements jlo..jhi accumulating into dense
            pcomp.  Element b = 16t+j lands in PE column-quadrant j%4, row
            4t+j//4 within it, so consecutive j cycle quadrants and run
            ~4-way concurrent (all-one-quadrant comp serialized at
            ~6.2us/tile).  Host unscrambles the row permutation."""
            w32g = s["w32g"]
            for j in range(jlo, jhi):
                qd = j % 4
                for s0, s1 in ((0, SPLIT), (SPLIT, N)):
                    nc.tensor.matmul(
                        pcomp[32 * qd:32 * qd + 32, s0:s1],
                        w32g[:, j, :],
                        emb_of(s, j)[:, s0:s1],
                        start=(t == 0 and j < 4),
                        stop=(t == NT - 1 and j >= TB - 4),
                        tile_position=(0, 32 * qd))

        def softmax_exp(t, pcm):
            exn = exn_pool.tile([128, NCH * 128], BF16, tag="exn")
            sums = sm_pool.tile([128, 1], F32, tag="sums")
            nc.scalar.activation(exn[:, :N], pcm[:, :N], AF.Exp,
                                 accum_out=sums[:])
            return exn, sums

        def softmax_recip(t, sums):
            # separate from the exp issue point so the DVE A_copy of the
            # previous tile isn't queued behind recip/diagb
            recip = sm_pool.tile([128, 1], F32, tag="recip")
            nc.vector.reciprocal(recip[:], sums[:])
            diagb = sm_pool.tile([128, 128], BF16, tag="diagb")
            nc.vector.tensor_scalar_mul(diagb[:], w_sb["identf"][:], recip[:])
            return diagb

        def transpose_tile(t, exn, diagb):
            """attnT [128, c, 128] bf16: normalized attn, node 8p+c at
            row p (rows 125..127 zero via the exn col padding)."""
            attnT = attnT_pool.tile([128, NCH, 128], BF16, tag="attnT")
            exn_v = exn[:].rearrange("r (p c) -> r c p", c=NCH)
            pat = pat_pool.tile([128, NCH, 128], F32, tag="pat")
            for c in range(NCH):
                nc.tensor.matmul(pat[:, c, :], exn_v[:, c, :], diagb[:],
                                 start=True, stop=True)
            # split the PSUM->SBUF copies across ACT and DVE so they run
            # in parallel (they are on the critical path to the A-pass)
            nc.scalar.copy(attnT[:, 0:4, :], pat[:, 0:4, :])
            nc.vector.tensor_copy(attnT[:, 4:8, :], pat[:, 4:8, :])
            return attnT

        def a_pass_pairs(t, natg, attnT, pA):
            """128 (LDW nat-chunk[128,128], 8-col MM) pairs, back-to-back;
            they pipeline at ~33ns/pair."""
            for j in range(TB):
                col0 = 32 * (j % 4) + 8 * (j // 4)
                for c in range(NCH):
                    nc.tensor.matmul(
                        pA[:, j * H:(j + 1) * H],
                        natg[:, j, c, :],
                        attnT[:, c, col0:col0 + H],
                        start=(c == 0), stop=(c == NCH - 1))

        def heads_part1(t, paux, pA):
            """pairs(t) -> A_sb (DVE copy) -> pw = sum_h M_h A[:,h] (8
            accumulating PE matmuls; replaces the old pheads/mask-reduce/pw
            chain and its two DVE round-trips)."""
            A_sb = sm_pool.tile([E, TB * H], BF16, tag="A")
            nc.vector.tensor_copy(A_sb[:], pA)
            A_v = A_sb[:].rearrange("e (j h) -> e h j", h=H)
            pw = paux[:, 256:272]
            for h in range(H):
                nc.tensor.matmul(pw, w_sb["mhcat"][:, h, :], A_v[:, h, :],
                                 start=(h == 0), stop=(h == H - 1))
            return pw

        def w_part2(t, paux, pw, w32g):
            """w32g[:, j, :] gets w_j at column 4t + j//4 (the comp row
            within element j's quadrant); issued one iteration later."""
            base = w32g[:]
            dst = bass.AP(base.tensor,
                          base.offset + 4 * t,
                          [list(base.ap[0]), [129, 4], [32, 4]])
            nc.scalar.copy(dst, pw.rearrange("e (jo ji) -> e jo ji", ji=4))

        # -------- software-pipelined main loop --------
        # Block i (clock = embT(i) half-b landing; one idle gap per block
        # at the end so HAM warms during the contiguous PE run):
        #   compat_b(i) | exp(i)[ACT] | comp(i-1) j<8 | pat(i) copies |
        #   comp(i-1) j>=8 | pairs(i) | Mh(i) w32copy(i) | compat_a(i+1)
        st = {}   # per-tile state dicts

        def compat_a(i, ea, eb, natg):
            w32g = sm_pool.tile([E, TB, 32], BF16, tag="w32g")
            nc.gpsimd.memset(w32g[:], 0.0)
            pcm = pcm_pool.tile([128, 1024], F32, tag="pcm")
            st[i] = dict(embTa=ea, embTb=eb, natg=natg, w32g=w32g, pcm=pcm)
            compat_mms(i, st[i], pcm, 0, 2)

        def compat_b(i):
            p = st[i]
            compat_mms(i, p, p["pcm"], 2, 4)
            exn, sums = softmax_exp(i, p["pcm"])
            p["exn"] = exn
            p["diagb"] = softmax_recip(i, sums)

        loaded = {0: preload0, 1: preload1}
        compat_a(0, *loaded[0])
        for i in range(NT + 1):
            if i < NT:
                compat_b(i)
            if i + 2 < NT:
                loaded[i + 2] = load_tile(i + 2)
            if i >= 1:
                comp_mms(i - 1, st[i - 1], 0, TB // 2)
            if i < NT:
                p = st[i]
                p["attnT"] = transpose_tile(i, p["exn"], p["diagb"])
            if i >= 1:
                comp_mms(i - 1, st[i - 1], TB // 2, TB)
            if i < NT:
                p = st[i]
                paux = paux_pool.tile([E, 512], F32, tag="paux")
                p["paux"] = paux
                p["pA"] = paux[:, 0:128]
                a_pass_pairs(i, p["natg"], p["attnT"], p["pA"])
                p["pw"] = heads_part1(i, p["paux"], p["pA"])
                w_part2(i, p["paux"], p["pw"], p["w32g"])
            if i + 1 < NT:
                compat_a(i + 1, *loaded[i + 1])

        # -------- epilogue: tanh, probs softmax (no-max), outputs --------
        nc.scalar.activation(t_th[:], pcomp[:, :N], AF.Tanh)
        nc.gpsimd.dma_start(tanh_out[:], t_th[:])
        sums2 = stage_pool.tile([128, 1], F32, tag="sums2")
        nc.scalar.activation(p_stage[:], t_th[:], AF.Exp, scale=10.0,
                             accum_out=sums2[:])
        recip2 = stage_pool.tile([128, 1], F32, tag="recip2")
        nc.vector.reciprocal(recip2[:], sums2[:])
        nc.vector.tensor_scalar_mul(p_stage[:], p_stage[:], recip2[:])
        nc.sync.dma_start(probs_out[:], p_stage[:])

    nc.compile()
    return nc


def _get_nc():
    key = (BC, N_CORES)
    if key not in _NC_CACHE:
        _NC_CACHE[key] = _build_nc(*key)
    return _NC_CACHE[key]


def kernel(embeddings, remaining_capacity, Wqg, Wkg, Wvg, Wog, Wqo, Wko,
           current_node, mask):
    global LAST_RESULT
    embeddings = np.asarray(embeddings, dtype=np.float32)
    remaining_capacity = np.asarray(remaining_capacity, dtype=np.float32)
    Wqg = np.asarray(Wqg, dtype=np.float32)
    Wkg = np.asarray(Wkg, dtype=np.float32)
    Wvg = np.asarray(Wvg, dtype=np.float32)
    Wog = np.asarray(Wog, dtype=np.float32)
    Wqo = np.asarray(Wqo, dtype=np.float32)
    Wko = np.asarray(Wko, dtype=np.float32)
    current_node = np.asarray(current_node).astype(np.int64)
    mask = np.asarray(mask)
    assert embeddings.shape == (B, N, E)

    trace = bool(os.environ.get("BASS_TRACE"))
    if trace:
        _install_profile_shim()

    w = _host_prep_weights(Wvg, Wog, Wqo, Wko)
    U32, cur = _host_prep_u32(embeddings, remaining_capacity, Wqg, Wkg,
                              current_node)
    emb_bf = embeddings.astype(BF16_NP)                        # [B, N, E]
    # embt [B/16=64, E, 16, N]: embt[bt, e, j, n] = emb[16bt+j, n, e] --
    # the on-chip embT layout, so a plain per-tile DMA has one contiguous
    # 32KB DRAM run per partition.
    embt = np.ascontiguousarray(
        emb_bf.transpose(2, 0, 1)                              # [E, B, N]
        .reshape(E, B // TB, TB, N).transpose(1, 0, 2, 3))     # [bt, E, TB, N]
    # natp [B/16, 128, 16, 8, E]: natp[bt, p, j, c, e] = emb[16bt+j, 8p+c, e]
    # (rows p>=125 zero) -- the on-chip glimpse-accumulation layout, fp8.
    nat_np = FP8_NP if NAT_FP8 else BF16_NP
    emb_nat = np.zeros((B, 128, NCH, E), dtype=nat_np)
    emb_nat[:, :CH] = embeddings.reshape(B, CH, NCH, E)
    natp = np.ascontiguousarray(
        emb_nat.reshape(B // TB, TB, 128, NCH, E).transpose(0, 2, 1, 3, 4))

    nc = _get_nc()
    in_maps = []
    for c in range(N_CORES):
        tl = slice(c * NT, (c + 1) * NT)
        sl = slice(c * BC, (c + 1) * BC)
        m = {
            "embt": embt[tl],
            "natp": natp[tl],
            "u32": np.ascontiguousarray(U32[:, sl].reshape(E, BC * 32)),
        }
        m.update(w)
        in_maps.append(m)

    kw = {}
    if trace:
        kw = dict(trace=True, trace_cores=[0])
    res = run_bass_kernel_spmd(nc, in_maps, list(range(N_CORES)), **kw)
    LAST_RESULT = res

    # device rows are permuted: element b=16t+j of a core sits at row
    # 32*(j%4) + 4t + j//4 (comp quadrant spread); invert per core.
    t_ = np.arange(BC) // TB
    j_ = np.arange(BC) % TB
    rho = 32 * (j_ % 4) + 4 * t_ + j_ // 4
    probs = np.concatenate(
        [res.results[c]["probs"][rho] for c in range(N_CORES)], 0)
    tanh = np.concatenate(
        [res.results[c]["tanh"][rho] for c in range(N_CORES)], 0)
    logits = 10.0 * tanh

    if mask.any():
        # General-correctness slow path (the spec always sends an all-False
        # mask): the mask affects the glimpse attention too, so recompute
        # everything for the masked rows on the host.
        probs, logits = _numpy_full(embeddings, remaining_capacity, Wqg, Wkg,
                                    Wvg, Wog, Wqo, Wko, cur, mask)

    return probs.astype(np.float32), logits.astype(np.float32)


def _numpy_full(emb, capv, Wqg, Wkg, Wvg, Wog, Wqo, Wko, cur, mask):
    graph = emb.mean(axis=1)
    context = np.concatenate([graph, cur, capv[:, None]], axis=-1)
    q = (context @ Wqg).reshape(B, H, D)
    k = (emb @ Wkg).reshape(B, N, H, D)
    v = (emb @ Wvg).reshape(B, N, H, D)
    compat = np.einsum('bhd,bnhd->bhn', q, k) / math.sqrt(D)
    compat = np.where(mask[:, None, :], -np.inf, compat)
    m = compat.max(axis=-1, keepdims=True)
    a = np.exp(compat - m)
    attn = a / a.sum(axis=-1, keepdims=True)
    heads = np.einsum('bhn,bnhd->bhd', attn, v).reshape(B, E)
    glimpse = heads @ Wog
    qo = glimpse @ Wqo
    ko = emb @ Wko
    comp = np.einsum('be,bne->bn', qo, ko) / math.sqrt(E)
    logits = 10.0 * np.tanh(comp)
    logits = np.where(mask, -np.inf, logits)
    m2 = logits.max(axis=-1, keepdims=True)
    a2 = np.exp(logits - m2)
    probs = a2 / a2.sum(axis=-1, keepdims=True)
    return probs.astype(np.float32), logits.astype(np.float32)

